# revision 15
# baseline (speedup 1.0000x reference)
"""CenterLossLayer Trainium2 kernel — 8-core SPMD, collective-free.

Math (reference):
    sel   = onehot @ centers                      # [B, D] — a row gather
    delta = onehot.T @ (sel - features)           # [C, D] — a scatter-add
    counts = onehot.sum(0) + 1                    # [C, 1]
    new_centers = centers - ALPHA * delta / counts
    loss = sum((features - sel)^2, axis=1)        # [B, 1]

Since row i of `onehot @ centers` is exactly centers[label_i]:
    delta = counts ⊙ centers − onehot.T @ features
    new_centers = centers·s1 + (onehot.T @ features)·s2,
        s1 = (1−ALPHA) + ALPHA/(counts+1),  s2 = ALPHA/(counts+1)

Sharding: pure CLASS sharding — core j owns classes [1250j, 1250j+1250) and
reads the matching onehot COLUMN slice [4096, 1250] (same total onehot
traffic as row sharding) plus the full features. Everything becomes local:
  * delta matmul: lhsT = bf16 cast of the onehot slice (no label decode at
    all), rhs = [features_bf16 | 1] chunks; ones column -> per-class counts.
  * loss: rows whose label falls in this shard are exactly the rows with a
    1 in the slice. Per 128-row chunk: bf16 max_index (row max is known to
    be 1.0) -> local index or a huge u32 on miss; sel is pre-filled with the
    row's own bf16 features, then a bounds-checked indirect gather
    overwrites matched rows with centers_l[idx] — so unmatched rows yield
    diff == 0 exactly. Each core emits a full-size partial loss vector
    (zeros off-shard); the host SUMS the 8 partials (each row is covered by
    exactly one core). No inter-core communication anywhere.
Engine split: ScalarE does the bf16 casts/pre-fills/squares, VectorE the
max_index scans and reductions, PE the 320 accumulating matmuls, GpSimd the
indirect gathers.
"""
import sys

import numpy as np

sys.path.insert(0, "/opt/trn_rl_repo")

import concourse.bass as bass  # noqa: E402
import concourse.tile as tile  # noqa: E402
from concourse import bacc, mybir  # noqa: E402
from concourse.bass import IndirectOffsetOnAxis  # noqa: E402
from concourse.bass_utils import run_bass_kernel_spmd  # noqa: E402

ALPHA = 0.5
B, C, D = 4096, 10000, 256
N_CORES = 8
CL = C // N_CORES          # 1250 classes per core
P = 128
NGBLK = B // P             # 32 row chunks of 128
EX = D + 1                 # rhs chunk row: feat | 1
F32 = mybir.dt.float32
BF16 = mybir.dt.bfloat16
U32 = mybir.dt.uint32
AX = mybir.AxisListType
OP = mybir.AluOpType
AF = mybir.ActivationFunctionType

_CACHE = {}


def _build():
    nc = bacc.Bacc("TRN2", target_bir_lowering=False, debug=False,
                   num_devices=N_CORES)
    oh_cols = nc.dram_tensor("oh_cols", [B, CL], F32,
                             kind="ExternalInput").ap()
    features_full = nc.dram_tensor("features_full", [B, D], F32,
                                   kind="ExternalInput").ap()
    centers_l = nc.dram_tensor("centers_l", [CL, D], F32,
                               kind="ExternalInput").ap()
    loss_p = nc.dram_tensor("loss_p", [B, 1], F32,
                            kind="ExternalOutput").ap()
    newc_l = nc.dram_tensor("newc_l", [CL, D], F32,
                            kind="ExternalOutput").ap()

    with tile.TileContext(nc) as tc:
        with tc.tile_pool(name="const", bufs=1) as constp, \
             tc.tile_pool(name="oh", bufs=3) as ohp, \
             tc.tile_pool(name="big", bufs=1) as bigp, \
             tc.tile_pool(name="fst", bufs=2) as fstp, \
             tc.tile_pool(name="ls", bufs=3) as lsp, \
             tc.tile_pool(name="upd", bufs=2) as updp, \
             tc.tile_pool(name="psum", bufs=4, space="PSUM") as psp:

            ones8 = constp.tile([P, 8], BF16, name="ones8")
            nc.vector.memset(ones8[:], 1.0)

            rhs_all = bigp.tile([P, NGBLK * EX], BF16, name="rhs_all")
            recon_all = bigp.tile([P, NGBLK * CL], BF16, name="recon_all")

            # ---- stage features -> bf16 rhs chunks [feat|1] ----
            for q in range(4):  # 8 chunks per DMA
                fst = fstp.tile([P, 8 * D], F32, tag="fst")
                src = features_full[q * 8 * P:(q + 1) * 8 * P, :]
                nc.sync.dma_start(
                    fst[:].rearrange("p (n d) -> p n d", n=8),
                    src.rearrange("(n p) d -> p n d", p=P))
                dst = rhs_all[:, q * 8 * EX:(q + 1) * 8 * EX]
                nc.vector.tensor_copy(
                    dst.rearrange("p (n e) -> p n e", n=8)[:, :, 0:D],
                    fst[:].rearrange("p (n d) -> p n d", n=8))
                nc.vector.memset(
                    dst.rearrange("p (n e) -> p n e", n=8)[:, :, D:EX], 1.0)

            # ---- per 128-row chunk: cast slice, scan, loss ----
            for g in range(NGBLK):
                rsl = slice(g * CL, (g + 1) * CL)
                oh = ohp.tile([P, CL], F32, tag="oh")
                nc.sync.dma_start(oh[:], oh_cols[g * P:(g + 1) * P, :])
                nc.scalar.activation(out=recon_all[:, rsl], in_=oh[:],
                                     func=AF.Copy)
                idx8 = lsp.tile([P, 8], U32, tag="idx8")
                nc.vector.max_index(idx8[:], ones8[:], recon_all[:, rsl])

                fsl = rhs_all[:, g * EX:g * EX + D]
                sel = lsp.tile([P, D], F32, tag="sel")
                nc.scalar.activation(out=sel[:], in_=fsl, func=AF.Copy)
                nc.gpsimd.indirect_dma_start(
                    out=sel[:], out_offset=None, in_=centers_l[:],
                    in_offset=IndirectOffsetOnAxis(ap=idx8[:, 0:1], axis=0),
                    bounds_check=CL - 1, oob_is_err=False)
                diff = lsp.tile([P, D], F32, tag="diff")
                nc.vector.tensor_sub(diff[:], sel[:], fsl)
                sq = lsp.tile([P, D], F32, tag="sq")
                nc.scalar.activation(out=sq[:], in_=diff[:], func=AF.Square)
                loss_r = lsp.tile([P, 1], F32, tag="loss_r")
                nc.vector.reduce_sum(loss_r[:], sq[:], axis=AX.X)
                # mask off rows whose label is outside this core's shard
                # (robust to either leave-untouched or zero-fill OOB-gather
                # semantics, and to stale SBUF in sel)
                mask = lsp.tile([P, 1], F32, tag="mask")
                nc.vector.tensor_scalar(out=mask[:], in0=idx8[:, 0:1],
                                        scalar1=CL - 1, scalar2=None,
                                        op0=OP.is_le)
                loss_t = lsp.tile([P, 1], F32, tag="loss_t")
                nc.vector.tensor_mul(loss_t[:], loss_r[:], mask[:])
                nc.sync.dma_start(loss_p[g * P:(g + 1) * P, :], loss_t[:])

            # ---- delta matmuls (m-outer) + center update ----
            mts = [(m0, min(P, CL - m0)) for m0 in range(0, CL, P)]
            for m0, msz in mts:
                ps = psp.tile([P, EX], F32, tag="ps", name=f"ps_{m0}")
                for g in range(NGBLK):
                    nc.tensor.matmul(
                        out=ps[:msz, :],
                        lhsT=recon_all[:, g * CL + m0:g * CL + m0 + msz],
                        rhs=rhs_all[:, g * EX:(g + 1) * EX],
                        start=(g == 0), stop=(g == NGBLK - 1))
                cnt1 = updp.tile([P, 1], F32, tag="cnt1")
                nc.vector.tensor_scalar_add(cnt1[:msz], ps[:msz, D:D + 1],
                                            1.0)
                recip = updp.tile([P, 1], F32, tag="recip")
                nc.vector.reciprocal(recip[:msz], cnt1[:msz])
                s2 = updp.tile([P, 1], F32, tag="s2")
                nc.vector.tensor_scalar_mul(s2[:msz], recip[:msz], ALPHA)
                s1 = updp.tile([P, 1], F32, tag="s1")
                nc.vector.tensor_scalar(out=s1[:msz], in0=recip[:msz],
                                        scalar1=ALPHA, scalar2=1.0 - ALPHA,
                                        op0=OP.mult, op1=OP.add)
                cen = updp.tile([P, D], F32, tag="cen")
                nc.sync.dma_start(cen[:msz], centers_l[m0:m0 + msz, :])
                t1 = updp.tile([P, D], F32, tag="t1")
                nc.scalar.activation(out=t1[:msz], in_=cen[:msz],
                                     func=AF.Copy, scale=s1[:msz, :1])
                t2 = updp.tile([P, D], F32, tag="t2")
                nc.vector.tensor_scalar(out=t2[:msz], in0=ps[:msz, 0:D],
                                        scalar1=s2[:msz, :1],
                                        scalar2=None, op0=OP.mult)
                newc = updp.tile([P, D], F32, tag="newc")
                nc.vector.tensor_add(newc[:msz], t1[:msz], t2[:msz])
                nc.sync.dma_start(newc_l[m0:m0 + msz, :], newc[:msz])
    nc.compile()
    return nc


def _get_nc():
    if "nc" not in _CACHE:
        _CACHE["nc"] = _build()
    return _CACHE["nc"]


def _in_maps(features, onehot, centers):
    return [{
        "oh_cols": np.ascontiguousarray(onehot[:, i * CL:(i + 1) * CL]),
        "features_full": features,
        "centers_l": centers[i * CL:(i + 1) * CL],
    } for i in range(N_CORES)]


def kernel(features, onehot, centers):
    features = np.ascontiguousarray(features, dtype=np.float32)
    onehot = np.ascontiguousarray(onehot, dtype=np.float32)
    centers = np.ascontiguousarray(centers, dtype=np.float32)
    nc = _get_nc()
    res = run_bass_kernel_spmd(nc, _in_maps(features, onehot, centers),
                               core_ids=list(range(N_CORES)))
    loss = np.sum([res.results[i]["loss_p"] for i in range(N_CORES)], axis=0)
    new_centers = np.concatenate(
        [res.results[i]["newc_l"] for i in range(N_CORES)], axis=0)
    return loss, new_centers


# revision 16
# speedup vs baseline: 1.7158x; 1.7158x over previous
"""CenterLossLayer Trainium2 kernel — 8-core SPMD, collective-free.

Math (reference):
    sel   = onehot @ centers                      # [B, D] — a row gather
    delta = onehot.T @ (sel - features)           # [C, D] — a scatter-add
    counts = onehot.sum(0) + 1                    # [C, 1]
    new_centers = centers - ALPHA * delta / counts
    loss = sum((features - sel)^2, axis=1)        # [B, 1]

Since row i of `onehot @ centers` is exactly centers[label_i]:
    delta = counts ⊙ centers − onehot.T @ features
    new_centers = centers·s1 + (onehot.T @ features)·s2,
        s1 = (1−ALPHA) + ALPHA/(counts+1),  s2 = ALPHA/(counts+1)

Sharding: pure CLASS sharding — core j owns classes [1250j, 1250j+1250) and
reads the matching onehot COLUMN slice [4096, 1250] (same total onehot
traffic as row sharding) plus the full features. No inter-core
communication at all:
  * delta matmul: lhsT = bf16 cast of the onehot slice (no label decode),
    rhs = [features_bf16 | 1] chunks; the ones column gives per-class
    counts over the whole batch.
  * loss: per 128-row chunk, DVE max_index over the f32 slice (row max is
    known to be 1.0) -> local class index, or huge u32 on miss; sel is
    pre-filled with the row's own bf16 features and a bounds-checked
    indirect gather overwrites matched rows with centers_l[idx], so
    unmatched rows give diff == 0 exactly (HW leaves OOB rows untouched;
    NB CoreSim zero-fills them instead, so sim shows a loss mismatch).
    Each core emits a full-size partial loss vector; the host SUMS the 8
    partials (each row is matched by exactly one core).
Engine split: ScalarE casts/squares, VectorE scans/reduces/prefills,
GpSimd gathers+subtracts, PE the 320 accumulating matmuls (8 class tiles
pipelined chunk-by-chunk in 8 PSUM banks, last 2 tiles as a short tail).
Stage-major program order keeps every engine queue free of mid-stream
long-latency waits.
"""
import sys

import numpy as np

sys.path.insert(0, "/opt/trn_rl_repo")

import concourse.bass as bass  # noqa: E402
import concourse.tile as tile  # noqa: E402
from concourse import bacc, mybir  # noqa: E402
from concourse.bass import IndirectOffsetOnAxis  # noqa: E402
from concourse.bass_utils import run_bass_kernel_spmd  # noqa: E402

ALPHA = 0.5
B, C, D = 4096, 10000, 256
N_CORES = 8
CL = C // N_CORES          # 1250 classes per core
P = 128
NGBLK = B // P             # 32 row chunks of 128
EX = D + 1                 # rhs chunk row: feat | 1
F32 = mybir.dt.float32
BF16 = mybir.dt.bfloat16
U32 = mybir.dt.uint32
AX = mybir.AxisListType
OP = mybir.AluOpType
AF = mybir.ActivationFunctionType

_CACHE = {}


def _build():
    nc = bacc.Bacc("TRN2", target_bir_lowering=False, debug=False,
                   num_devices=N_CORES)
    oh_cols = nc.dram_tensor("oh_cols", [B, CL], F32,
                             kind="ExternalInput").ap()
    features_full = nc.dram_tensor("features_full", [B, D], F32,
                                   kind="ExternalInput").ap()
    centers_l = nc.dram_tensor("centers_l", [CL, D], F32,
                               kind="ExternalInput").ap()
    loss_p = nc.dram_tensor("loss_p", [B, 1], F32,
                            kind="ExternalOutput").ap()
    newc_l = nc.dram_tensor("newc_l", [CL, D], F32,
                            kind="ExternalOutput").ap()

    with tile.TileContext(nc) as tc:
        with tc.tile_pool(name="const", bufs=1) as constp, \
             tc.tile_pool(name="oh", bufs=4) as ohp, \
             tc.tile_pool(name="big", bufs=1) as bigp, \
             tc.tile_pool(name="fst", bufs=2) as fstp, \
             tc.tile_pool(name="sel", bufs=8) as selp, \
             tc.tile_pool(name="df", bufs=4) as dfp, \
             tc.tile_pool(name="upd", bufs=2) as updp, \
             tc.tile_pool(name="psum", bufs=8, space="PSUM") as psp:

            ones8 = constp.tile([P, 8], F32, name="ones8")
            nc.vector.memset(ones8[:], 1.0)

            rhs_all = bigp.tile([P, NGBLK * EX], BF16, name="rhs_all")
            recon_all = bigp.tile([P, NGBLK * CL], BF16, name="recon_all")
            idx_all = bigp.tile([P, NGBLK * 8], U32, name="idx_all")
            loss_all = bigp.tile([P, NGBLK], F32, name="loss_all")

            # ---- stage features -> bf16 rhs chunks [feat|1] ----
            for q in range(4):  # 8 chunks per DMA
                fst = fstp.tile([P, 8 * D], F32, tag="fst")
                src = features_full[q * 8 * P:(q + 1) * 8 * P, :]
                nc.sync.dma_start(
                    fst[:].rearrange("p (n d) -> p n d", n=8),
                    src.rearrange("(n p) d -> p n d", p=P))
                dst = rhs_all[:, q * 8 * EX:(q + 1) * 8 * EX]
                nc.vector.tensor_copy(
                    dst.rearrange("p (n e) -> p n e", n=8)[:, :, 0:D],
                    fst[:].rearrange("p (n d) -> p n d", n=8))
                nc.vector.memset(
                    dst.rearrange("p (n e) -> p n e", n=8)[:, :, D:EX], 1.0)

            # ---- per-chunk front half: DMA, bf16 cast, scan, prefill ----
            sels = {}
            for g in range(NGBLK):
                rsl = slice(g * CL, (g + 1) * CL)
                oh = ohp.tile([P, CL], F32, tag="oh")
                nc.sync.dma_start(oh[:], oh_cols[g * P:(g + 1) * P, :])
                nc.scalar.activation(out=recon_all[:, rsl], in_=oh[:],
                                     func=AF.Copy)
                nc.vector.max_index(idx_all[:, 8 * g:8 * g + 8], ones8[:],
                                    oh[:])
                sel = selp.tile([P, D], F32, tag="sel", name=f"sel{g}")
                nc.vector.tensor_copy(sel[:], rhs_all[:, g * EX:g * EX + D])
                sels[g] = sel

            # ---- per-chunk back half: gather + diff (GpSimd), loss ----
            for g in range(NGBLK):
                sel = sels[g]
                nc.gpsimd.indirect_dma_start(
                    out=sel[:], out_offset=None, in_=centers_l[:],
                    in_offset=IndirectOffsetOnAxis(
                        ap=idx_all[:, 8 * g:8 * g + 1], axis=0),
                    bounds_check=CL - 1, oob_is_err=False)
                diff = dfp.tile([P, D], F32, tag="diff")
                nc.gpsimd.tensor_sub(diff[:], sel[:],
                                     rhs_all[:, g * EX:g * EX + D])
                sq = dfp.tile([P, D], F32, tag="sq")
                nc.scalar.activation(out=sq[:], in_=diff[:], func=AF.Square)
                nc.vector.reduce_sum(loss_all[:, g:g + 1], sq[:], axis=AX.X)
            nc.sync.dma_start(
                loss_p.rearrange("(g p) o -> p g o", p=P)[:, :, 0],
                loss_all[:])

            # ---- delta matmuls: 8 class-tiles pipelined, 2 as tail ----
            mts = [(m0, min(P, CL - m0)) for m0 in range(0, CL, P)]
            ps = [psp.tile([P, EX], F32, tag="ps", name=f"ps_{i}")
                  for i in range(len(mts))]
            for g in range(NGBLK):
                for i in range(8):
                    m0, msz = mts[i]
                    nc.tensor.matmul(
                        out=ps[i][:msz, :],
                        lhsT=recon_all[:, g * CL + m0:g * CL + m0 + msz],
                        rhs=rhs_all[:, g * EX:(g + 1) * EX],
                        start=(g == 0), stop=(g == NGBLK - 1))
            for g in range(NGBLK):
                for i in (8, 9):
                    m0, msz = mts[i]
                    nc.tensor.matmul(
                        out=ps[i][:msz, :],
                        lhsT=recon_all[:, g * CL + m0:g * CL + m0 + msz],
                        rhs=rhs_all[:, g * EX:(g + 1) * EX],
                        start=(g == 0), stop=(g == NGBLK - 1))

            # ---- update: newc = centers*s1 + mm*s2 ----
            for i, (m0, msz) in enumerate(mts):
                cnt1 = updp.tile([P, 1], F32, tag="cnt1")
                nc.vector.tensor_scalar_add(cnt1[:msz], ps[i][:msz, D:D + 1],
                                            1.0)
                recip = updp.tile([P, 1], F32, tag="recip")
                nc.vector.reciprocal(recip[:msz], cnt1[:msz])
                s2 = updp.tile([P, 1], F32, tag="s2")
                nc.vector.tensor_scalar_mul(s2[:msz], recip[:msz], ALPHA)
                s1 = updp.tile([P, 1], F32, tag="s1")
                nc.vector.tensor_scalar(out=s1[:msz], in0=recip[:msz],
                                        scalar1=ALPHA, scalar2=1.0 - ALPHA,
                                        op0=OP.mult, op1=OP.add)
                cen = updp.tile([P, D], F32, tag="cen")
                nc.sync.dma_start(cen[:msz], centers_l[m0:m0 + msz, :])
                t1 = updp.tile([P, D], F32, tag="t1")
                nc.scalar.activation(out=t1[:msz], in_=cen[:msz],
                                     func=AF.Copy, scale=s1[:msz, :1])
                t2 = updp.tile([P, D], F32, tag="t2")
                nc.vector.tensor_scalar(out=t2[:msz], in0=ps[i][:msz, 0:D],
                                        scalar1=s2[:msz, :1],
                                        scalar2=None, op0=OP.mult)
                newc = updp.tile([P, D], F32, tag="newc")
                nc.vector.tensor_add(newc[:msz], t1[:msz], t2[:msz])
                nc.sync.dma_start(newc_l[m0:m0 + msz, :], newc[:msz])
    nc.compile()
    return nc


def _get_nc():
    if "nc" not in _CACHE:
        _CACHE["nc"] = _build()
    return _CACHE["nc"]


def _in_maps(features, onehot, centers):
    return [{
        "oh_cols": np.ascontiguousarray(onehot[:, i * CL:(i + 1) * CL]),
        "features_full": features,
        "centers_l": centers[i * CL:(i + 1) * CL],
    } for i in range(N_CORES)]


def kernel(features, onehot, centers):
    features = np.ascontiguousarray(features, dtype=np.float32)
    onehot = np.ascontiguousarray(onehot, dtype=np.float32)
    centers = np.ascontiguousarray(centers, dtype=np.float32)
    nc = _get_nc()
    res = run_bass_kernel_spmd(nc, _in_maps(features, onehot, centers),
                               core_ids=list(range(N_CORES)))
    loss = np.sum([res.results[i]["loss_p"] for i in range(N_CORES)], axis=0)
    new_centers = np.concatenate(
        [res.results[i]["newc_l"] for i in range(N_CORES)], axis=0)
    return loss, new_centers
